# revision 4
# baseline (speedup 1.0000x reference)
"""Trainium2 Bass kernel for nn_Conv3x3 (3x3 conv, stride 3 == kernel, no overlap).

Math (reduced from the switched-capacitor reference):
    out[w, h] = -(1/0.924458) * sum_{i,j} x[3w+i, 3h+j] * weight[i, j]
    returned flattened to (2048*2048,), row-major over (w, h).

Strategy: the whole reduction runs on the TensorEngine. For each column
phase j in {0,1,2}, a band-structured weight matrix (built on host from
the 9 weight values, output scale folded in) is the stationary operand
and the x tile with a stride-3 column access pattern is the moving
operand; the three phases accumulate in PSUM. ScalarE copies PSUM->SBUF
and DMA stores to the output.

Sharding (8 cores): rows 0..6047 are 48 blocks of 126 rows; core c takes
6 blocks = x rows [756c, 756c+756) -> out rows [252c, 252c+252). The
last 96 x rows (out rows 2016..2047) are split column-wise: core c takes
out cols [256c, 256c+256) of them. This keeps every PE block full-width
(PE cost is per streamed column, independent of K).

Self-contained: hardcodes shapes/sharding for x=(6144,6144) f32,
weight=(3,3) f32 on 8 NeuronCores.
"""

import numpy as np

INIT_C1_SCALED = 0.924458
SCALE = -1.0 / INIT_C1_SCALED

NCORES = 8
SIDE = 6144                   # x is (SIDE, SIDE)
OUT_SIDE = SIDE // 3          # 2048

BLK_ROWS = 126                # x rows per main block
BLK_OUT = BLK_ROWS // 3       # 42
N_CHUNK = 512                 # fp32 moving-operand max
N_CHUNKS = OUT_SIDE // N_CHUNK  # 4

MAIN_BLOCKS = 6               # per core
MAIN_ROWS = MAIN_BLOCKS * BLK_ROWS        # 756 x rows per core
MAIN_OUT = MAIN_BLOCKS * BLK_OUT          # 252 out rows per core

REM_X0 = NCORES * MAIN_ROWS   # 6048: first x row of the shared remainder
REM_ROWS = SIDE - REM_X0      # 96
REM_OUT = REM_ROWS // 3       # 32 out rows (2016..2047)
REM_COLS = OUT_SIDE // NCORES  # 256 out cols per core
REM_XCOLS = 3 * REM_COLS      # 768 x cols per core

USE_F32R = False              # reduced-precision fast matmul mode

_PREPARED = {}


def _split_excess_waits(nc, max_main=1):
    """walrus in this env rejects >1 sync wait per instruction; spill
    extras onto same-engine NoOps placed immediately before."""
    from concourse import mybir

    counter = 0
    for f in nc.m.functions:
        for bb in f.blocks:
            new = []
            changed = False
            for ins in bb.instructions:
                si = ins.sync_info
                waits = list(si.on_wait) if si and si.on_wait else []
                if len(waits) > max_main:
                    for w in waits[:-max_main]:
                        nop = mybir.InstNoOp(name=f"I-wsplit-{counter}")
                        counter += 1
                        nop.engine = ins.engine
                        nop.sync_info = mybir.SyncInfo(on_wait=[w], on_update=[])
                        new.append(nop)
                    ins.sync_info = mybir.SyncInfo(
                        on_wait=waits[-max_main:],
                        on_update=list(si.on_update) if si.on_update else [],
                    )
                    changed = True
                new.append(ins)
            if changed:
                bb.instructions = new


def build_program(reps=1, use_f32r=None):
    """Build the SPMD Bass program (one NeuronCore's slab). reps>1 repeats
    the body for marginal-timing runs."""
    import concourse.bass as bass
    import concourse.tile as tile
    from concourse import mybir

    if use_f32r is None:
        use_f32r = USE_F32R
    f32 = mybir.dt.float32
    f32r = mybir.dt.float32r
    mmdt = f32r if use_f32r else f32

    nc = bass.Bass("TRN2", target_bir_lowering=False, debug=False)
    xs = nc.dram_tensor("xs", [MAIN_ROWS, SIDE], f32, kind="ExternalInput").ap()
    xm = nc.dram_tensor("xm", [REM_ROWS, REM_XCOLS], f32, kind="ExternalInput").ap()
    wb = nc.dram_tensor("wb", [BLK_ROWS, 3 * BLK_OUT], f32, kind="ExternalInput").ap()
    out = nc.dram_tensor("out", [MAIN_OUT, OUT_SIDE], f32, kind="ExternalOutput").ap()
    out2 = nc.dram_tensor("out2", [REM_OUT, REM_COLS], f32, kind="ExternalOutput").ap()

    with tile.TileContext(nc) as tc:
        with (
            tc.tile_pool(name="wpool", bufs=1) as wpool,
            tc.tile_pool(name="xpool", bufs=4) as xpool,
            tc.tile_pool(name="opool", bufs=3) as opool,
            tc.tile_pool(name="pspool", bufs=2, space="PSUM") as pspool,
        ):
            wt_raw = wpool.tile([BLK_ROWS, 3 * BLK_OUT], f32)
            nc.sync.dma_start(wt_raw[:], wb)
            if use_f32r:
                wt = wpool.tile([BLK_ROWS, 3 * BLK_OUT], f32r)
                nc.scalar.copy(wt[:], wt_raw[:])
            else:
                wt = wt_raw

            for _ in range(reps):
                for b in range(MAIN_BLOCKS):
                    xt_raw = xpool.tile([BLK_ROWS, SIDE], f32, tag="xt")
                    nc.sync.dma_start(
                        xt_raw[:], xs[b * BLK_ROWS : (b + 1) * BLK_ROWS, :]
                    )
                    if use_f32r:
                        xt = xpool.tile([BLK_ROWS, SIDE], f32r, tag="xtr")
                        nc.vector.tensor_copy(xt[:], xt_raw[:])
                    else:
                        xt = xt_raw
                    pt = pspool.tile([BLK_OUT, OUT_SIDE], f32, tag="pt")
                    for c in range(N_CHUNKS):
                        base = 3 * N_CHUNK * c
                        for j in range(3):
                            nc.tensor.matmul(
                                pt[:, c * N_CHUNK : (c + 1) * N_CHUNK],
                                wt[:, j * BLK_OUT : (j + 1) * BLK_OUT],
                                xt[:, base + j : base + j + 3 * (N_CHUNK - 1) + 1 : 3],
                                start=(j == 0),
                                stop=(j == 2),
                            )
                    ot = opool.tile([BLK_OUT, OUT_SIDE], f32, tag="ot")
                    nc.scalar.copy(ot[:], pt[:])
                    nc.sync.dma_start(
                        out[b * BLK_OUT : (b + 1) * BLK_OUT, :], ot[:]
                    )

                # shared-remainder mini block: 96 rows x 768 cols -> (32, 256)
                mt_raw = xpool.tile([REM_ROWS, REM_XCOLS], f32, tag="mt")
                nc.sync.dma_start(mt_raw[:], xm)
                if use_f32r:
                    mt = xpool.tile([REM_ROWS, REM_XCOLS], f32r, tag="mtr")
                    nc.vector.tensor_copy(mt[:], mt_raw[:])
                else:
                    mt = mt_raw
                pm = pspool.tile([REM_OUT, REM_COLS], f32, tag="pt")
                for j in range(3):
                    nc.tensor.matmul(
                        pm[:],
                        wt[0:REM_ROWS, j * BLK_OUT : j * BLK_OUT + REM_OUT],
                        mt[:, j : j + 3 * (REM_COLS - 1) + 1 : 3],
                        start=(j == 0),
                        stop=(j == 2),
                    )
                om = opool.tile([REM_OUT, REM_COLS], f32, tag="om")
                nc.scalar.copy(om[:], pm[:])
                nc.sync.dma_start(out2[:], om[:])

    _split_excess_waits(nc)
    return nc


def build_wband(weight):
    """wband[p, 42*j + w'] = SCALE * weight[p%3, j] if p//3 == w' else 0."""
    wband = np.zeros((BLK_ROWS, 3 * BLK_OUT), np.float32)
    w = np.asarray(weight, dtype=np.float32)
    for p in range(BLK_ROWS):
        i, wp = p % 3, p // 3
        for j in range(3):
            wband[p, BLK_OUT * j + wp] = SCALE * w[i, j]
    return wband


def make_in_maps(x, weight):
    x = np.ascontiguousarray(np.asarray(x, dtype=np.float32))
    assert x.shape == (SIDE, SIDE)
    wband = build_wband(weight)
    maps = []
    for c in range(NCORES):
        maps.append(
            {
                "xs": x[c * MAIN_ROWS : (c + 1) * MAIN_ROWS],
                "xm": np.ascontiguousarray(
                    x[REM_X0:, c * REM_XCOLS : (c + 1) * REM_XCOLS]
                ),
                "wb": wband,
            }
        )
    return maps


def assemble_output(results):
    out = np.empty((OUT_SIDE, OUT_SIDE), np.float32)
    for c in range(NCORES):
        out[c * MAIN_OUT : (c + 1) * MAIN_OUT, :] = results[c]["out"]
        out[NCORES * MAIN_OUT :, c * REM_COLS : (c + 1) * REM_COLS] = results[c]["out2"]
    return out.reshape(-1)


def kernel(x, weight):
    from concourse.bass_utils import run_bass_kernel_spmd

    if "nc" not in _PREPARED:
        _PREPARED["nc"] = build_program()
    nc = _PREPARED["nc"]

    in_maps = make_in_maps(x, weight)
    res = run_bass_kernel_spmd(nc, in_maps, list(range(NCORES)))
    return assemble_output(res.results)


# revision 5
# speedup vs baseline: 1.6578x; 1.6578x over previous
"""Trainium2 Bass kernel for nn_Conv3x3 (3x3 conv, stride 3 == kernel, no overlap).

Math (reduced from the switched-capacitor reference):
    out[w, h] = -(1/0.924458) * sum_{i,j} x[3w+i, 3h+j] * weight[i, j]
    returned flattened to (2048*2048,), row-major over (w, h).

Strategy: the whole reduction runs on the TensorEngine. For each column
phase j in {0,1,2}, a band-structured weight matrix (built on host from
the 9 weight values, output scale folded in) is the stationary operand
and the x tile with a stride-3 column access pattern is the moving
operand; the three phases accumulate in PSUM. ScalarE copies PSUM->SBUF
and DMA stores to the output.

Sharding (8 cores): rows 0..6047 are 48 blocks of 126 rows; core c takes
6 blocks = x rows [756c, 756c+756) -> out rows [252c, 252c+252). The
last 96 x rows (out rows 2016..2047) are split column-wise: core c takes
out cols [256c, 256c+256) of them. This keeps every PE block full-width
(PE cost is per streamed column, independent of K).

Self-contained: hardcodes shapes/sharding for x=(6144,6144) f32,
weight=(3,3) f32 on 8 NeuronCores.
"""

import numpy as np

INIT_C1_SCALED = 0.924458
SCALE = -1.0 / INIT_C1_SCALED

NCORES = 8
SIDE = 6144                   # x is (SIDE, SIDE)
OUT_SIDE = SIDE // 3          # 2048

BLK_ROWS = 126                # x rows per main block
BLK_OUT = BLK_ROWS // 3       # 42
N_CHUNK = 512                 # fp32 moving-operand max
N_CHUNKS = OUT_SIDE // N_CHUNK  # 4

MAIN_BLOCKS = 6               # per core
MAIN_ROWS = MAIN_BLOCKS * BLK_ROWS        # 756 x rows per core
MAIN_OUT = MAIN_BLOCKS * BLK_OUT          # 252 out rows per core

REM_X0 = NCORES * MAIN_ROWS   # 6048: first x row of the shared remainder
REM_ROWS = SIDE - REM_X0      # 96
REM_OUT = REM_ROWS // 3       # 32 out rows (2016..2047)
REM_COLS = OUT_SIDE // NCORES  # 256 out cols per core
REM_XCOLS = 3 * REM_COLS      # 768 x cols per core

USE_F32R = False              # reduced-precision fast matmul mode

_PREPARED = {}


def _split_excess_waits(nc, max_main=1):
    """walrus in this env rejects >1 sync wait per instruction; spill
    extras onto same-engine NoOps placed immediately before."""
    from concourse import mybir

    counter = 0
    for f in nc.m.functions:
        for bb in f.blocks:
            new = []
            changed = False
            for ins in bb.instructions:
                si = ins.sync_info
                waits = list(si.on_wait) if si and si.on_wait else []
                if len(waits) > max_main:
                    for w in waits[:-max_main]:
                        nop = mybir.InstNoOp(name=f"I-wsplit-{counter}")
                        counter += 1
                        nop.engine = ins.engine
                        nop.sync_info = mybir.SyncInfo(on_wait=[w], on_update=[])
                        new.append(nop)
                    ins.sync_info = mybir.SyncInfo(
                        on_wait=waits[-max_main:],
                        on_update=list(si.on_update) if si.on_update else [],
                    )
                    changed = True
                new.append(ins)
            if changed:
                bb.instructions = new


def build_program(reps=1, use_f32r=None):
    """Build the SPMD Bass program (one NeuronCore's slab). reps>1 repeats
    the body for marginal-timing runs."""
    import concourse.bass as bass
    import concourse.tile as tile
    from concourse import mybir

    if use_f32r is None:
        use_f32r = USE_F32R
    f32 = mybir.dt.float32
    f32r = mybir.dt.float32r
    mmdt = f32r if use_f32r else f32

    nc = bass.Bass("TRN2", target_bir_lowering=False, debug=False)
    xs = nc.dram_tensor("xs", [MAIN_ROWS, SIDE], f32, kind="ExternalInput").ap()
    xm = nc.dram_tensor("xm", [REM_ROWS, REM_XCOLS], f32, kind="ExternalInput").ap()
    wb = nc.dram_tensor("wb", [BLK_ROWS, 3 * BLK_OUT], f32, kind="ExternalInput").ap()
    out = nc.dram_tensor("out", [MAIN_OUT, OUT_SIDE], f32, kind="ExternalOutput").ap()
    out2 = nc.dram_tensor("out2", [REM_OUT, REM_COLS], f32, kind="ExternalOutput").ap()

    with tile.TileContext(nc) as tc:
        with (
            tc.tile_pool(name="wpool", bufs=1) as wpool,
            tc.tile_pool(name="xpool", bufs=4) as xpool,
            tc.tile_pool(name="opool", bufs=3) as opool,
            tc.tile_pool(name="pspool", bufs=2, space="PSUM") as pspool,
        ):
            wt_raw = wpool.tile([BLK_ROWS, 3 * BLK_OUT], f32)
            nc.sync.dma_start(wt_raw[:], wb)
            if use_f32r:
                wt = wpool.tile([BLK_ROWS, 3 * BLK_OUT], f32r)
                nc.scalar.copy(wt[:], wt_raw[:])
            else:
                wt = wt_raw

            for _ in range(reps):
                for b in range(MAIN_BLOCKS):
                    xt_raw = xpool.tile([BLK_ROWS, SIDE], f32, tag="xt", bufs=2 if use_f32r else None)
                    nc.sync.dma_start(
                        xt_raw[:], xs[b * BLK_ROWS : (b + 1) * BLK_ROWS, :]
                    )
                    if use_f32r:
                        xt = xpool.tile([BLK_ROWS, SIDE], f32r, tag="xtr", bufs=2)
                        nc.vector.tensor_copy(xt[:], xt_raw[:])
                    else:
                        xt = xt_raw
                    pt = pspool.tile([BLK_OUT, OUT_SIDE], f32, tag="pt")
                    for c in range(N_CHUNKS):
                        base = 3 * N_CHUNK * c
                        for j in range(3):
                            nc.tensor.matmul(
                                pt[:, c * N_CHUNK : (c + 1) * N_CHUNK],
                                wt[:, j * BLK_OUT : (j + 1) * BLK_OUT],
                                xt[:, base + j : base + j + 3 * (N_CHUNK - 1) + 1 : 3],
                                start=(j == 0),
                                stop=(j == 2),
                            )
                    ot = opool.tile([BLK_OUT, OUT_SIDE], f32, tag="ot")
                    nc.scalar.copy(ot[:], pt[:])
                    nc.sync.dma_start(
                        out[b * BLK_OUT : (b + 1) * BLK_OUT, :], ot[:]
                    )

                # shared-remainder mini block: 96 rows x 768 cols -> (32, 256)
                mt_raw = xpool.tile([REM_ROWS, REM_XCOLS], f32, tag="mt")
                nc.sync.dma_start(mt_raw[:], xm)
                if use_f32r:
                    mt = xpool.tile([REM_ROWS, REM_XCOLS], f32r, tag="mtr")
                    nc.vector.tensor_copy(mt[:], mt_raw[:])
                else:
                    mt = mt_raw
                pm = pspool.tile([REM_OUT, REM_COLS], f32, tag="pt")
                for j in range(3):
                    nc.tensor.matmul(
                        pm[:],
                        wt[0:REM_ROWS, j * BLK_OUT : j * BLK_OUT + REM_OUT],
                        mt[:, j : j + 3 * (REM_COLS - 1) + 1 : 3],
                        start=(j == 0),
                        stop=(j == 2),
                    )
                om = opool.tile([REM_OUT, REM_COLS], f32, tag="om")
                nc.scalar.copy(om[:], pm[:])
                nc.sync.dma_start(out2[:], om[:])

    _split_excess_waits(nc)
    return nc


def build_wband(weight):
    """wband[p, 42*j + w'] = SCALE * weight[p%3, j] if p//3 == w' else 0."""
    wband = np.zeros((BLK_ROWS, 3 * BLK_OUT), np.float32)
    w = np.asarray(weight, dtype=np.float32)
    for p in range(BLK_ROWS):
        i, wp = p % 3, p // 3
        for j in range(3):
            wband[p, BLK_OUT * j + wp] = SCALE * w[i, j]
    return wband


def make_in_maps(x, weight):
    x = np.ascontiguousarray(np.asarray(x, dtype=np.float32))
    assert x.shape == (SIDE, SIDE)
    wband = build_wband(weight)
    maps = []
    for c in range(NCORES):
        maps.append(
            {
                "xs": x[c * MAIN_ROWS : (c + 1) * MAIN_ROWS],
                "xm": np.ascontiguousarray(
                    x[REM_X0:, c * REM_XCOLS : (c + 1) * REM_XCOLS]
                ),
                "wb": wband,
            }
        )
    return maps


def assemble_output(results):
    out = np.empty((OUT_SIDE, OUT_SIDE), np.float32)
    for c in range(NCORES):
        out[c * MAIN_OUT : (c + 1) * MAIN_OUT, :] = results[c]["out"]
        out[NCORES * MAIN_OUT :, c * REM_COLS : (c + 1) * REM_COLS] = results[c]["out2"]
    return out.reshape(-1)


def kernel(x, weight):
    from concourse.bass_utils import run_bass_kernel_spmd

    if "nc" not in _PREPARED:
        _PREPARED["nc"] = build_program()
    nc = _PREPARED["nc"]

    in_maps = make_in_maps(x, weight)
    res = run_bass_kernel_spmd(nc, in_maps, list(range(NCORES)))
    return assemble_output(res.results)
